# revision 3
# baseline (speedup 1.0000x reference)
"""CostVolumeLayer3D Trainium2 kernel.

Computes cv[b, ch, d, y, x] = (1/125) * sum_c x1[b,c,d,y,x] * x2[b,c,d-h,y-i,x-j]
for the 45 channels that survive the reference's channel-collapse
(ch = (5*(i+j)+h) % 125, last write in (i,j,h) loop order wins -> for each
diagonal s=i+j the winner is i=min(2,s+2), j=s-i). Remaining 80 channels are 0.

Sharding: depth D=32 split across 8 cores (4 output slices each); the host
supplies each core a zero/halo-padded x2 shard so every shifted window is a
plain strided view.

Per-core layout: SBUF partitions = (b, c) = 2*64 = 128. Free axis = padded
(d', y', x') volume of x2, so a 3D shift is a free-axis offset view.
DVE computes shifted elementwise products (fp16, 2x mode); PE reduces over
the 64 channels via one-hot fp16 matmuls accumulating all 45 shifts into
PSUM rows 0..89 = (shift, b); ACT extracts with the 1/125 scale to fp32.
"""

import numpy as np

_B, _C, _D, _H, _W = 2, 64, 32, 64, 64
_R = 2
_NCH = 125
_NCORES = 8
_DL = _D // _NCORES          # output depth slices per core (4)
_DH = _DL + 2 * _R           # x2 depth slices incl. halo (8)
_YB = 32                     # y block
_NYB = _H // _YB             # 2
_YH = _YB + 2 * _R           # 36
_XH = _W + 2 * _R            # 68
_COMPUTE_DT = "float16"      # on-device product dtype
_NFREE = _YB * _W            # free elems per tile (2048)
_MMN = 512                   # matmul moving free dim


def _shift_table():
    """45 surviving shifts as (out_channel, d_off, y_off, x_off) where the
    x2 window for output (t, y, x) starts at padded index
    (t + d_off, y + y_off, x + x_off)."""
    shifts = []
    for sd in range(-4, 5):
        i = min(2, sd + 2)
        j = sd - i
        for h in range(-2, 3):
            shifts.append(((5 * sd + h) % _NCH, _R - h, _R - i, _R - j))
    return shifts


_SHIFTS = _shift_table()
_NS = len(_SHIFTS)           # 45
_M = 2 * _NS                 # psum rows: (shift, b)


def _ones_lhst(np_dt):
    """One matmul weight matrix per shift: lhsT[k, s, m] routes the partition
    half k//64 (= batch) of shift s's products to psum row 2*s + k//64."""
    a = np.zeros((128, _NS, _M), dtype=np_dt)
    for s in range(_NS):
        a[0:64, s, 2 * s] = 1.0
        a[64:128, s, 2 * s + 1] = 1.0
    return a


_prog = None


def _build_program():
    global _prog
    if _prog is not None:
        return _prog
    from contextlib import ExitStack

    import concourse.bacc as bacc
    import concourse.mybir as mybir
    import concourse.tile as tile

    dt_in = getattr(mybir.dt, _COMPUTE_DT)
    f32 = mybir.dt.float32
    nc = bacc.Bacc(trn_type="TRN2", debug=False)
    x1_d = nc.dram_tensor("x1", [_B, _C, _DL, _H, _W], dt_in, kind="ExternalInput")
    x2_d = nc.dram_tensor(
        "x2", [_B, _C, _DH, _H + 2 * _R, _XH], dt_in, kind="ExternalInput"
    )
    on_d = nc.dram_tensor("ones", [128, _NS, _M], dt_in, kind="ExternalInput")
    out_d = nc.dram_tensor("out", [_NS, _B, _DL, _H, _W], f32, kind="ExternalOutput")

    with tile.TileContext(nc) as tc:
        with ExitStack() as ctx:
            constp = ctx.enter_context(tc.tile_pool(name="const", bufs=1))
            x2p = ctx.enter_context(tc.tile_pool(name="x2res", bufs=1))
            x2op = ctx.enter_context(tc.tile_pool(name="x2odd", bufs=1))
            x1p = ctx.enter_context(tc.tile_pool(name="x1", bufs=3))
            prodp = ctx.enter_context(tc.tile_pool(name="prod", bufs=4))
            psump = ctx.enter_context(tc.tile_pool(name="psum", bufs=2, space="PSUM"))
            stagep = ctx.enter_context(tc.tile_pool(name="stage", bufs=2))

            ones_t = constp.tile([128, _NS, _M], dt_in)
            nc.sync.dma_start(ones_t[:], on_d.ap())

            for yh in range(_NYB):
                y0 = yh * _YB
                x2_t = x2p.tile([128, _DH, _YH, _XH], dt_in)
                nc.sync.dma_start(
                    x2_t[:],
                    x2_d.ap()[:, :, :, y0 : y0 + _YH, :].rearrange(
                        "b c d y x -> (b c) d y x"
                    ),
                )
                # x-shift-by-one copy keeps odd-j windows 4B-aligned (DVE 2x mode)
                x2o_t = x2op.tile([128, _DH, _YH, _XH], dt_in)
                nc.scalar.copy(x2o_t[:, :, :, 0 : _XH - 1], x2_t[:, :, :, 1:_XH])
                for t in range(_DL):
                    x1_t = x1p.tile([128, _YB, _W], dt_in)
                    nc.sync.dma_start(
                        x1_t[:],
                        x1_d.ap()[:, :, t, y0 : y0 + _YB, :].rearrange(
                            "b c y x -> (b c) y x"
                        ),
                    )
                    ps = psump.tile([128, _NFREE], f32)
                    for s, (_ch, dd0, yy0, xx0) in enumerate(_SHIFTS):
                        pr = prodp.tile([128, _YB, _W], dt_in)
                        dv = t + dd0
                        if xx0 % 2 == 0:
                            xv = x2_t[:, dv, yy0 : yy0 + _YB, xx0 : xx0 + _W]
                        else:
                            xv = x2o_t[:, dv, yy0 : yy0 + _YB, xx0 - 1 : xx0 - 1 + _W]
                        nc.vector.tensor_mul(pr[:], x1_t[:], xv)
                        prf = pr[:].rearrange("p y x -> p (y x)")
                        for n in range(_NFREE // _MMN):
                            nc.tensor.matmul(
                                ps[0:_M, _MMN * n : _MMN * (n + 1)],
                                lhsT=ones_t[:, s, :],
                                rhs=prf[:, _MMN * n : _MMN * (n + 1)],
                                start=(s == 0),
                                stop=(s == _NS - 1),
                            )
                    st = stagep.tile([128, _NFREE], f32)
                    nc.scalar.mul(st[0:_M, :], ps[0:_M, :], 1.0 / _NCH)
                    nc.sync.dma_start(
                        out_d.ap()[:, :, t, y0 : y0 + _YB, :].rearrange(
                            "s b y x -> (s b) (y x)"
                        ),
                        st[0:_M, :],
                    )
    nc.compile()
    _prog = nc
    return nc


def _np_dt():
    return np.float16 if _COMPUTE_DT == "float16" else np.float32


def _shard_inputs(x1, x2):
    np_dt = _np_dt()
    x2pad = np.pad(
        np.asarray(x2), ((0, 0), (0, 0), (_R, _R), (_R, _R), (_R, _R))
    ).astype(np_dt)
    x1 = np.asarray(x1)
    ones_np = _ones_lhst(np_dt)
    in_maps = []
    for k in range(_NCORES):
        d0 = k * _DL
        in_maps.append(
            {
                "x1": np.ascontiguousarray(x1[:, :, d0 : d0 + _DL].astype(np_dt)),
                "x2": np.ascontiguousarray(x2pad[:, :, d0 : d0 + _DH]),
                "ones": ones_np,
            }
        )
    return in_maps


def _gather(results):
    out = np.zeros((_B, _NCH, _D, _H, _W), dtype=np.float32)
    for k in range(_NCORES):
        o = results[k]["out"]  # [45, B, DL, H, W] fp32
        d0 = k * _DL
        for s, (ch, _dd0, _yy0, _xx0) in enumerate(_SHIFTS):
            out[:, ch, d0 : d0 + _DL] = o[s]
    return out


def _run(in_maps, **kwargs):
    from concourse.bass_utils import run_bass_kernel_spmd

    nc = _build_program()
    return run_bass_kernel_spmd(nc, in_maps, core_ids=list(range(_NCORES)), **kwargs)


def kernel(**inputs):
    res = _run(_shard_inputs(inputs["x1"], inputs["x2"]))
    return _gather(res.results)
